# revision 1
# baseline (speedup 1.0000x reference)
"""Block-sparse attention (sliding window of 3 x 64-token blocks) on 8 trn2 cores.

Problem: B=1, H=16, S=4096, D=128, fp32 I/O. Token i attends to token j iff
|i//64 - j//64| <= 1, i.e. a 192-key window per 64-query block.

Sharding: head-parallel — 2 heads per NeuronCore, no cross-core traffic.

Per-core kernel (per head):
  - Host packs, per head, ONE fp16 input tensor in consumption order:
    5 chunks, each [qT cols | kT cols | augmented-V cols], so one DMA per
    chunk streams everything the next batch of q-tiles needs.
      qT: Q^T [d=128, S]
      kT: K^T zero-padded by 64 keys on each end [d=128, S+128]
      va: V augmented with a ones-column, rearranged to [128, 33*129] so
          each 128-key chunk c lives at cols [129c, 129c+129)
  - For each 128-query tile t (2 blocks), the allowed keys are the padded
    window [128t, 128t+256): two 128-key chunks A=t, B=t+1.
  - MM1 (PE):  S^T = K_chunk @ Q_tile^T -> PSUM [keys=128, q=128] per chunk
               (lhsT = kT slice, rhs = qT slice; contraction over d).
  - ACT:       P = exp(S^T * 1/sqrt(d)) -> SBUF fp16 (no max-subtraction:
               scores are ~N(0,1), |s| < ~6, exact softmax up to rounding).
  - GPSIMD:    memset the one disallowed 64x64 corner per chunk to 0
               (keys block 2t-1 can't serve q block 2t+1 and vice versa);
               boundary tiles zero the full 64-row pad block instead.
  - MM2 (PE):  psO [q=128, 129] = P_A^T.T @ VA + P_B^T.T @ VB accumulated in
               PSUM; col 128 (the ones-column) accumulates the softmax
               denominator for free.
  - DVE:       plain copy psO -> fp32 SBUF (normalization division happens
               on the HOST: out = PV/den — removes 2 chain stages).
  - Output written as [128, 16 pairs * 258] (partition = q-within-tile,
    per pair [PV_A|den_A|PV_B|den_B]); host divides and reassembles.

Emission is software-pipelined: pair n+1's MM1 block is emitted before pair
n's exp/memset/MM2/copy so the scheduler keeps the ACT engine (the measured
steady-state rate limiter, ~0.65us exp per pair) continuously fed.
"""

import bisect
import math

import numpy as np

B, H, S, D = 1, 16, 4096, 128
N_CORES = 8
HPC = H // N_CORES          # heads per core
TILE = 128
NT = S // TILE              # 32 query tiles per head
PAD = 64
SPAD = S + 2 * PAD          # 4224 padded keys
NCHUNK = SPAD // TILE       # 33 key chunks
VAW = NCHUNK * (D + 1)      # 4257 cols of rearranged augmented V
SCALE = 1.0 / math.sqrt(D)

# Packed-input chunking (consumption order; first chunk small so the first
# q-pair's dependencies land early). Boundaries are 128-aligned (qt/kt) and
# 129-aligned (va) so every kernel slice stays inside one segment.
_T = [0, 4, 8, 12, 16, 20, 24, 28, 32]   # q-tile boundaries per chunk
QT_B = [128 * t for t in _T]
KT_B = [0] + [128 * t + 256 for t in _T[1:-1]] + [SPAD]
VA_B = [0] + [129 * (t + 1) for t in _T[1:-1]] + [VAW]
NCK = len(QT_B) - 1
QT_W = [QT_B[i + 1] - QT_B[i] for i in range(NCK)]
KT_W = [KT_B[i + 1] - KT_B[i] for i in range(NCK)]
VA_W = [VA_B[i + 1] - VA_B[i] for i in range(NCK)]
CHUNK_W = [QT_W[i] + KT_W[i] + VA_W[i] for i in range(NCK)]
BASE = [0]
for i in range(NCK):
    BASE.append(BASE[-1] + CHUNK_W[i])
W_PACK = BASE[-1]

_PROGRAM = None


def _qt_off(x):
    i = bisect.bisect_right(QT_B, x) - 1
    return BASE[i] + (x - QT_B[i]), i


def _kt_off(y):
    i = bisect.bisect_right(KT_B, y) - 1
    return BASE[i] + QT_W[i] + (y - KT_B[i]), i


def _va_off(z):
    i = bisect.bisect_right(VA_B, z) - 1
    return BASE[i] + QT_W[i] + KT_W[i] + (z - VA_B[i]), i


def _build_program():
    from contextlib import ExitStack

    import concourse.mybir as mybir
    import concourse.tile as tile
    from concourse import bacc

    f16 = mybir.dt.float16
    f32 = mybir.dt.float32
    Exp = mybir.ActivationFunctionType.Exp

    nc = bacc.Bacc("TRN2", target_bir_lowering=False, debug=False)
    qkv_d = nc.declare_dram_parameter("qkv", [HPC, 128, W_PACK], f16, isOutput=False)
    out_d = nc.declare_dram_parameter("out", [HPC, 128, NT // 2 * 258], f32, isOutput=True)

    def qt_sl(sb, x0, w):
        off, i = _qt_off(x0)
        assert x0 + w <= QT_B[i + 1], (x0, w)
        return sb[:, off:off + w]

    def kt_sl(sb, y0, w):
        off, i = _kt_off(y0)
        assert y0 + w <= KT_B[i + 1], (y0, w)
        return sb[:, off:off + w]

    def va_sl(sb, z0, w):
        off, i = _va_off(z0)
        assert z0 + w <= VA_B[i + 1], (z0, w)
        return sb[:, off:off + w]

    with tile.TileContext(nc) as tc, ExitStack() as ctx:
        io_pool = ctx.enter_context(tc.tile_pool(name="io", bufs=2))
        out_pool = ctx.enter_context(tc.tile_pool(name="outp", bufs=2))
        p_pool = ctx.enter_context(tc.tile_pool(name="p", bufs=8))
        ps_pool = ctx.enter_context(tc.tile_pool(name="ps", bufs=4, space="PSUM"))
        po_pool = ctx.enter_context(tc.tile_pool(name="po", bufs=3, space="PSUM"))
        tch_pool = ctx.enter_context(tc.tile_pool(name="tch", bufs=1, space="PSUM"))
        tch = tch_pool.tile([1, 512], f32, tag="tch")

        # PE warmup sized to fit INSIDE the pre-data window (~7.3-9.2us:
        # after the framework preamble, before the first input chunk lands):
        # sustained PE activity flips the HAM clock gate to 2.4 GHz early and
        # reliably. Runs of this binary without warmup vary 44.5-49.5us, the
        # slow ones showing HAM-cold matmuls (median 76ns vs 59ns) stretching
        # the whole pipeline cadence.
        warm_pool = ctx.enter_context(tc.tile_pool(name="warm", bufs=1))
        warm = warm_pool.tile([128, 512], f16, tag="warm")
        nc.gpsimd.memset(warm[:], 0.0)
        for _ in range(4):
            nc.tensor.matmul(
                tch[:], lhsT=warm[:, 0:1], rhs=warm[:], start=True, stop=True
            )

        # Load phase: ALL input DMAs (both heads) are emitted first so they
        # outrank output DMAs in scheduler priority — otherwise head 0's
        # output chunks preempt head 1's input stream and head 1's compute
        # tail runs bandwidth-starved.
        io_sbs = []
        for h in range(HPC):
            io_sb = io_pool.tile([128, W_PACK], f16, tag="io")
            io_sbs.append(io_sb)
        for h in range(HPC):
            io_sb = io_sbs[h]
            for c in range(NCK):
                nc.sync.dma_start(
                    io_sb[:, BASE[c]:BASE[c + 1]], qkv_d[h, :, BASE[c]:BASE[c + 1]]
                )
                # PE "touch" of the freshly-loaded chunk: a 1-col dummy matmul
                # makes PE observe the DMA semaphore here, so the real matmuls
                # below stay within the 2-sync-wait HW limit.
                nc.tensor.matmul(
                    tch[0:1, 0:1], lhsT=io_sb[:, BASE[c]:BASE[c] + 1],
                    rhs=io_sb[:, BASE[c]:BASE[c] + 1], start=True, stop=True,
                )

        # Compute phase, software-pipelined EMISSION order: the MM1 block of
        # pair n+1 is emitted before pair n's exp/memset/MM2/copy, giving it
        # higher scheduler priority — the PE then produces scores one pair
        # ahead and the ACT engine (the steady-state rate limiter) never
        # starves. Pairs are flattened across heads so the head transition
        # pipelines too.
        pairs = [(h, u) for h in range(HPC) for u in range(NT // 2)]
        out_sbs = {}
        ps_tiles = {}

        def emit_mm1(h, u):
            io_sb = io_sbs[h]
            ps = ps_pool.tile([128, 512], f32, tag="ps")
            ps_tiles[(h, u)] = ps
            for j in range(2):
                t = 2 * u + j
                q_sl = qt_sl(io_sb, t * 128, 128)
                nc.tensor.matmul(
                    ps[:, 256 * j:256 * j + 128],
                    lhsT=kt_sl(io_sb, t * 128, 128),
                    rhs=q_sl, start=True, stop=True,
                )
                nc.tensor.matmul(
                    ps[:, 256 * j + 128:256 * j + 256],
                    lhsT=kt_sl(io_sb, t * 128 + 128, 128),
                    rhs=q_sl, start=True, stop=True,
                )

        def emit_tail(h, u):
            io_sb = io_sbs[h]
            out_sb = out_sbs[h]
            ps = ps_tiles.pop((h, u))
            p_sb = p_pool.tile([128, 512], f16, tag="p")
            nc.scalar.activation(p_sb[:], ps[:], Exp, bias=0.0, scale=SCALE)
            # Kill disallowed 64x64 corners (cols 256j+0:128 = chunk A of
            # tile t, 256j+128:256 = chunk B); boundary tiles kill the
            # whole 64-row pad block.
            for j in range(2):
                t = 2 * u + j
                c0 = 256 * j
                if t == 0:
                    nc.gpsimd.memset(p_sb[0:64, c0:c0 + 128], 0.0)
                else:
                    nc.gpsimd.memset(p_sb[0:64, c0 + 64:c0 + 128], 0.0)
                if t == NT - 1:
                    nc.gpsimd.memset(p_sb[64:128, c0 + 128:c0 + 256], 0.0)
                else:
                    nc.gpsimd.memset(p_sb[64:128, c0 + 128:c0 + 192], 0.0)
            po = po_pool.tile([128, 2 * (D + 1)], f32, tag="po")
            for j in range(2):
                t = 2 * u + j
                o0 = (D + 1) * j
                nc.tensor.matmul(
                    po[:, o0:o0 + D + 1],
                    lhsT=p_sb[:, 256 * j:256 * j + 128],
                    rhs=va_sl(io_sb, 129 * t, 129),
                    start=True, stop=False,
                )
                nc.tensor.matmul(
                    po[:, o0:o0 + D + 1],
                    lhsT=p_sb[:, 256 * j + 128:256 * j + 256],
                    rhs=va_sl(io_sb, 129 * (t + 1), 129),
                    start=False, stop=True,
                )
            nc.vector.tensor_copy(out_sb[:, u * 258:(u + 1) * 258], po[:])
            # Stream the output back in 4-pair chunks as they complete; the
            # last head's tail goes in 2-pair chunks so the final DMA (which
            # trails the last pair's compute) is shorter.
            last = h == HPC - 1 and u >= NT // 2 - 3
            if last:
                if u == 13:
                    c0, c1 = 12 * 258, 14 * 258
                    nc.sync.dma_start(out_d[h, :, c0:c1], out_sb[:, c0:c1])
                elif u >= 14:
                    c0, c1 = u * 258, (u + 1) * 258
                    nc.sync.dma_start(out_d[h, :, c0:c1], out_sb[:, c0:c1])
            elif u % 4 == 3:
                c0, c1 = (u - 3) * 258, (u + 1) * 258
                nc.sync.dma_start(out_d[h, :, c0:c1], out_sb[:, c0:c1])

        DEPTH = 1
        for n in range(len(pairs) + DEPTH):
            if n < len(pairs):
                h, u = pairs[n]
                if u == 0:
                    out_sb = out_pool.tile([128, NT // 2 * 258], f32, tag="out")
                    out_sbs[h] = out_sb
                emit_mm1(h, u)
            if n >= DEPTH:
                emit_tail(*pairs[n - DEPTH])

    nc.finalize()
    return nc


def _get_program():
    global _PROGRAM
    if _PROGRAM is None:
        _PROGRAM = _build_program()
    return _PROGRAM


def _pack_inputs(q, k, v):
    """q,k,v: [H, S, D] fp32 -> packed [H, 128, W_PACK] fp16 per head."""
    qt = np.ascontiguousarray(q.transpose(0, 2, 1)).astype(np.float16)  # [H,128,S]
    k_pad = np.zeros((H, SPAD, D), np.float32)
    k_pad[:, PAD:PAD + S] = k
    kt = np.ascontiguousarray(k_pad.transpose(0, 2, 1)).astype(np.float16)
    v_aug = np.zeros((H, SPAD, D + 1), np.float32)
    v_aug[:, PAD:PAD + S, :D] = v
    v_aug[:, :, D] = 1.0
    va = np.ascontiguousarray(
        v_aug.reshape(H, NCHUNK, 128, D + 1).transpose(0, 2, 1, 3)
    ).reshape(H, 128, VAW).astype(np.float16)
    segs = []
    for c in range(NCK):
        segs.append(qt[:, :, QT_B[c]:QT_B[c + 1]])
        segs.append(kt[:, :, KT_B[c]:KT_B[c + 1]])
        segs.append(va[:, :, VA_B[c]:VA_B[c + 1]])
    return np.ascontiguousarray(np.concatenate(segs, axis=2))


def kernel(q, k, v):
    """q, k, v: [1, 16, 4096, 128] float32 -> [1, 16, 4096, 128] float32."""
    from concourse.bass_utils import run_bass_kernel_spmd

    q = np.asarray(q, dtype=np.float32).reshape(H, S, D)
    k = np.asarray(k, dtype=np.float32).reshape(H, S, D)
    v = np.asarray(v, dtype=np.float32).reshape(H, S, D)

    qkv = _pack_inputs(q, k, v)
    in_maps = [
        {"qkv": np.ascontiguousarray(qkv[c * HPC:(c + 1) * HPC])}
        for c in range(N_CORES)
    ]

    nc = _get_program()
    results = run_bass_kernel_spmd(nc, in_maps, list(range(N_CORES))).results

    out = np.empty((H, S, D), np.float32)
    for c in range(N_CORES):
        o = results[c]["out"]  # [HPC, 128, 16*258] = per pair [PV_A|den_A|PV_B|den_B]
        for j in range(HPC):
            x = o[j].reshape(128, NT, D + 1)        # [p, t, 129]
            pv = x[:, :, :D] / x[:, :, D:D + 1]     # normalize on host
            out[c * HPC + j] = pv.transpose(1, 0, 2).reshape(S, D)
    return out.reshape(B, H, S, D)



# revision 2
# speedup vs baseline: 1.0972x; 1.0972x over previous
"""Block-sparse attention (sliding window of 3 x 64-token blocks) on 8 trn2 cores.

Problem: B=1, H=16, S=4096, D=128, fp32 I/O. Token i attends to token j iff
|i//64 - j//64| <= 1, i.e. a 192-key window per 64-query block.

Sharding: head-parallel - 2 heads per NeuronCore, no cross-core traffic.

v2 (from trace analysis of the 45.9us baseline, which was DMA-paced at
~920ns/pair with ACT at 687ns/pair):
  - fp16 output (PV|den pairs; host divides in fp32): output DMA halves,
    4.23->2.11 MB/core. Total DMA 10.7->8.6 MB at ~390 GB/s sustained.
  - exp batched over a QUAD (2 pairs, 4 tiles): one ACTIVATE over
    [128,1024] PSUM (2 banks) instead of two over [128,512]; amortizes the
    ~180ns per-instruction PSUM-access + seq overhead: 687->~515ns/pair.
  - parity-swapped key-chunk layout: 128-key chunk c of kt/va stores its
    two 64-blocks as [hi|lo] for even c, [lo|hi] for odd c. Then for every
    q-tile t the two disallowed 64x64 corners land in the SAME partition
    half and ADJACENT columns -> ONE memset [64,128] per tile (2/pair
    instead of 4), uint32-bitcast to halve the free size. Boundary tiles
    just widen the rectangle.
  - PSUM: 3 quad-score buffers (3 x 2 banks) + 2 per-pair PV buffers
    (1 bank each) fill all 8 banks; emission runs DEPTH=2 quads ahead.
    TimelineSim: ps_bufs=2/DEPTH=1 serializes MM1(n+2) behind the full
    1us exp(n) PSUM read (37.2us modeled); 3 bufs + DEPTH=2 models 28.9.
  - PE warmup + per-chunk DMA "touch" matmuls write into a recycled ps
    pool tile instead of a dedicated PSUM scratch: frees 1 PSUM bank.

Per-quad chain: MM1 x8 (PE) -> exp [128,1024] (ACT) -> memset x4 (GPSIMD)
-> MM2 x8 (PE, accum pairs) -> copy x2 (DVE) -> out DMA every 2 quads.
Emission is software-pipelined two quads ahead (MM1 of quads n+1, n+2
before the tail of quad n) so the PE produces scores ahead and ACT never
starves. Steady-state budgets/pair: DMA ~685ns (pacer), ACT ~515, PE ~480,
DVE ~430, GPSIMD ~350.
"""

import bisect
import math

import numpy as np

B, H, S, D = 1, 16, 4096, 128
N_CORES = 8
HPC = H // N_CORES          # heads per core
TILE = 128
NT = S // TILE              # 32 query tiles per head
NQ = NT // 4                # 8 quads (4 tiles = 2 pairs) per head
PAD = 64
SPAD = S + 2 * PAD          # 4224 padded keys
NCHUNK = SPAD // TILE       # 33 key chunks
VAW = NCHUNK * (D + 1)      # 4257 cols of rearranged augmented V
OUT_W = NT * (D + 1)        # 4128 fp16 output cols per head
SCALE = 1.0 / math.sqrt(D)

# Packed-input chunking (consumption order; chunks are quad-aligned since a
# whole quad's MM1 is the first consumer). Boundaries are 128-aligned (qt/kt)
# and 129-aligned (va) so every kernel slice stays inside one segment.
_T = [0, 4, 8, 12, 16, 20, 24, 28, 32]   # q-tile boundaries per chunk
QT_B = [128 * t for t in _T]
KT_B = [0] + [128 * t + 256 for t in _T[1:-1]] + [SPAD]
VA_B = [0] + [129 * (t + 1) for t in _T[1:-1]] + [VAW]
NCK = len(QT_B) - 1
QT_W = [QT_B[i + 1] - QT_B[i] for i in range(NCK)]
KT_W = [KT_B[i + 1] - KT_B[i] for i in range(NCK)]
VA_W = [VA_B[i + 1] - VA_B[i] for i in range(NCK)]
CHUNK_W = [QT_W[i] + KT_W[i] + VA_W[i] for i in range(NCK)]
BASE = [0]
for i in range(NCK):
    BASE.append(BASE[-1] + CHUNK_W[i])
W_PACK = BASE[-1]

_PROGRAM = None


def _qt_off(x):
    i = bisect.bisect_right(QT_B, x) - 1
    return BASE[i] + (x - QT_B[i]), i


def _kt_off(y):
    i = bisect.bisect_right(KT_B, y) - 1
    return BASE[i] + QT_W[i] + (y - KT_B[i]), i


def _va_off(z):
    i = bisect.bisect_right(VA_B, z) - 1
    return BASE[i] + QT_W[i] + KT_W[i] + (z - VA_B[i]), i


def _build_program():
    from contextlib import ExitStack

    import concourse.mybir as mybir
    import concourse.tile as tile
    from concourse import bacc

    f16 = mybir.dt.float16
    f32 = mybir.dt.float32
    u32 = mybir.dt.uint32
    Exp = mybir.ActivationFunctionType.Exp

    nc = bacc.Bacc("TRN2", target_bir_lowering=False, debug=False)
    qkv_d = nc.declare_dram_parameter("qkv", [HPC, 128, W_PACK], f16, isOutput=False)
    out_d = nc.declare_dram_parameter("out", [HPC, 128, OUT_W], f16, isOutput=True)

    def qt_sl(sb, x0, w):
        off, i = _qt_off(x0)
        assert x0 + w <= QT_B[i + 1], (x0, w)
        return sb[:, off:off + w]

    def kt_sl(sb, y0, w):
        off, i = _kt_off(y0)
        assert y0 + w <= KT_B[i + 1], (y0, w)
        return sb[:, off:off + w]

    def va_sl(sb, z0, w):
        off, i = _va_off(z0)
        assert z0 + w <= VA_B[i + 1], (z0, w)
        return sb[:, off:off + w]

    with tile.TileContext(nc) as tc, ExitStack() as ctx:
        io_pool = ctx.enter_context(tc.tile_pool(name="io", bufs=2))
        out_pool = ctx.enter_context(tc.tile_pool(name="outp", bufs=2))
        p_pool = ctx.enter_context(tc.tile_pool(name="p", bufs=4))
        ps_pool = ctx.enter_context(tc.tile_pool(name="ps", bufs=3, space="PSUM"))
        po_pool = ctx.enter_context(tc.tile_pool(name="po", bufs=2, space="PSUM"))

        # PE warmup sized to fit INSIDE the pre-data window (~7.3-9.2us:
        # after the framework preamble, before the first input chunk lands):
        # sustained PE activity flips the HAM clock gate to 2.4 GHz early and
        # reliably. Runs without warmup vary 44.5-49.5us, the slow ones
        # showing HAM-cold matmuls stretching the whole pipeline cadence.
        # Warmup and the DMA "touch" matmuls below write into a recycled ps
        # pool tile (later quads overwrite it with start=True; same-engine
        # WAW keeps ordering) so no dedicated PSUM scratch bank is needed.
        warm_pool = ctx.enter_context(tc.tile_pool(name="warm", bufs=1))
        warm = warm_pool.tile([128, 512], f16, tag="warm")
        nc.gpsimd.memset(warm[:], 0.0)
        ps0 = ps_pool.tile([128, 1024], f32, tag="ps")
        for _ in range(4):
            nc.tensor.matmul(
                ps0[0:1, 0:512], lhsT=warm[:, 0:1], rhs=warm[:], start=True, stop=True
            )

        # Load phase: ALL input DMAs (both heads) are emitted first so they
        # outrank output DMAs in scheduler priority - otherwise head 0's
        # output chunks preempt head 1's input stream and head 1's compute
        # tail runs bandwidth-starved. Issued from Sync (HWDGE; gpsimd
        # dma_start would be SWDGE and starvable by DVE port locks).
        io_sbs = []
        for h in range(HPC):
            io_sb = io_pool.tile([128, W_PACK], f16, tag="io")
            io_sbs.append(io_sb)
        for h in range(HPC):
            io_sb = io_sbs[h]
            for c in range(NCK):
                nc.sync.dma_start(
                    io_sb[:, BASE[c]:BASE[c + 1]], qkv_d[h, :, BASE[c]:BASE[c + 1]]
                )
                # PE "touch" of the freshly-loaded chunk: a 1-col dummy matmul
                # makes PE observe the DMA semaphore here, so the real matmuls
                # below stay within the 2-sync-wait HW limit.
                nc.tensor.matmul(
                    ps0[0:1, 0:1], lhsT=io_sb[:, BASE[c]:BASE[c] + 1],
                    rhs=io_sb[:, BASE[c]:BASE[c] + 1], start=True, stop=True,
                )

        # Compute phase over quads (4 tiles), software-pipelined EMISSION
        # order: the MM1 block of quad n+1 is emitted before quad n's
        # exp/memset/MM2/copy, giving it higher scheduler priority - the PE
        # then produces scores one quad ahead and the ACT engine never
        # starves. Quads are flattened across heads so the head transition
        # pipelines too.
        quads = [(h, u) for h in range(HPC) for u in range(NQ)]
        out_sbs = {}
        ps_tiles = {}

        def emit_mm1(h, u):
            io_sb = io_sbs[h]
            ps = ps_pool.tile([128, 1024], f32, tag="ps")
            ps_tiles[(h, u)] = ps
            for j in range(4):
                t = 4 * u + j
                col = 256 * j
                q_sl = qt_sl(io_sb, t * 128, 128)
                nc.tensor.matmul(
                    ps[:, col:col + 128],
                    lhsT=kt_sl(io_sb, t * 128, 128),
                    rhs=q_sl, start=True, stop=True,
                )
                nc.tensor.matmul(
                    ps[:, col + 128:col + 256],
                    lhsT=kt_sl(io_sb, t * 128 + 128, 128),
                    rhs=q_sl, start=True, stop=True,
                )

        def emit_tail(h, u):
            io_sb = io_sbs[h]
            out_sb = out_sbs[h]
            ps = ps_tiles.pop((h, u))
            p_sb = p_pool.tile([128, 1024], f16, tag="p")
            nc.scalar.activation(p_sb[:], ps[:], Exp, bias=0.0, scale=SCALE)
            # Kill the disallowed corners. With the parity-swapped chunk
            # layout both 64x64 corners of tile t sit in one partition half
            # (even t: keys rows 64:128, odd t: rows 0:64) at cols
            # [64,192) of the tile's 256-col region; boundary tiles widen
            # the rectangle to also kill the pad block. uint32 bitcast
            # halves the free size GPSIMD has to walk.
            pu = p_sb.bitcast(u32)  # [128, 512]
            for j in range(4):
                t = 4 * u + j
                rows = slice(64, 128) if t % 2 == 0 else slice(0, 64)
                a, b = (0, 96) if t == 0 else ((32, 128) if t == NT - 1 else (32, 96))
                nc.gpsimd.memset(pu[rows, 128 * j + a:128 * j + b], 0)
            for jp in range(2):
                po = po_pool.tile([128, 258], f32, tag="po", name="po")
                for j in range(2):
                    t = 4 * u + 2 * jp + j
                    c0 = 512 * jp + 256 * j
                    o0 = 129 * j
                    nc.tensor.matmul(
                        po[:, o0:o0 + 129],
                        lhsT=p_sb[:, c0:c0 + 128],
                        rhs=va_sl(io_sb, 129 * t, 129),
                        start=True, stop=False,
                    )
                    nc.tensor.matmul(
                        po[:, o0:o0 + 129],
                        lhsT=p_sb[:, c0 + 128:c0 + 256],
                        rhs=va_sl(io_sb, 129 * (t + 1), 129),
                        start=False, stop=True,
                    )
                nc.vector.tensor_copy(
                    out_sb[:, 516 * u + 258 * jp:516 * u + 258 * (jp + 1)], po[:]
                )
            # Stream the output back in 2-quad chunks as they complete; the
            # last head's tail goes in 1-quad chunks so the final DMA (which
            # trails the last quad's compute) is shorter.
            last = h == HPC - 1 and u >= NQ - 2
            if last:
                c0, c1 = u * 516, (u + 1) * 516
                nc.sync.dma_start(out_d[h, :, c0:c1], out_sb[:, c0:c1])
            elif u % 2 == 1:
                c0, c1 = (u - 1) * 516, (u + 1) * 516
                nc.sync.dma_start(out_d[h, :, c0:c1], out_sb[:, c0:c1])

        DEPTH = 2
        for n in range(len(quads) + DEPTH):
            if n < len(quads):
                h, u = quads[n]
                if u == 0:
                    out_sb = out_pool.tile([128, OUT_W], f16, tag="out")
                    out_sbs[h] = out_sb
                emit_mm1(h, u)
            if n >= DEPTH:
                emit_tail(*quads[n - DEPTH])

    nc.finalize()
    return nc


def _get_program():
    global _PROGRAM
    if _PROGRAM is None:
        _PROGRAM = _build_program()
    return _PROGRAM


def _parity_swap(x):
    """x: [H, NCHUNK, 128, W] -> swap the two 64-row blocks of even chunks."""
    y = x.reshape(x.shape[0], NCHUNK, 2, 64, x.shape[-1])
    y = y.copy()
    y[:, 0::2] = y[:, 0::2, ::-1]
    return y.reshape(x.shape[0], NCHUNK, 128, x.shape[-1])


def _pack_inputs(q, k, v):
    """q,k,v: [H, S, D] fp32 -> packed [H, 128, W_PACK] fp16 per head."""
    qt = np.ascontiguousarray(q.transpose(0, 2, 1)).astype(np.float16)  # [H,128,S]
    k_pad = np.zeros((H, SPAD, D), np.float32)
    k_pad[:, PAD:PAD + S] = k
    k_pad = _parity_swap(k_pad.reshape(H, NCHUNK, 128, D)).reshape(H, SPAD, D)
    kt = np.ascontiguousarray(k_pad.transpose(0, 2, 1)).astype(np.float16)
    v_aug = np.zeros((H, SPAD, D + 1), np.float32)
    v_aug[:, PAD:PAD + S, :D] = v
    v_aug[:, :, D] = 1.0
    v_aug = _parity_swap(v_aug.reshape(H, NCHUNK, 128, D + 1))
    va = np.ascontiguousarray(
        v_aug.transpose(0, 2, 1, 3)
    ).reshape(H, 128, VAW).astype(np.float16)
    segs = []
    for c in range(NCK):
        segs.append(qt[:, :, QT_B[c]:QT_B[c + 1]])
        segs.append(kt[:, :, KT_B[c]:KT_B[c + 1]])
        segs.append(va[:, :, VA_B[c]:VA_B[c + 1]])
    return np.ascontiguousarray(np.concatenate(segs, axis=2))


def _unpack_outputs(results):
    out = np.empty((H, S, D), np.float32)
    for c in range(N_CORES):
        o = results[c]["out"]  # [HPC, 128, 32*129] fp16: per tile [PV|den]
        for j in range(HPC):
            x = o[j].reshape(128, NT, D + 1).astype(np.float32)
            pv = x[:, :, :D] / x[:, :, D:D + 1]     # normalize on host
            out[c * HPC + j] = pv.transpose(1, 0, 2).reshape(S, D)
    return out.reshape(B, H, S, D)


def kernel(q, k, v):
    """q, k, v: [1, 16, 4096, 128] float32 -> [1, 16, 4096, 128] float32."""
    from concourse.bass_utils import run_bass_kernel_spmd

    q = np.asarray(q, dtype=np.float32).reshape(H, S, D)
    k = np.asarray(k, dtype=np.float32).reshape(H, S, D)
    v = np.asarray(v, dtype=np.float32).reshape(H, S, D)

    qkv = _pack_inputs(q, k, v)
    in_maps = [
        {"qkv": np.ascontiguousarray(qkv[c * HPC:(c + 1) * HPC])}
        for c in range(N_CORES)
    ]

    nc = _get_program()
    results = run_bass_kernel_spmd(nc, in_maps, list(range(N_CORES))).results
    return _unpack_outputs(results)
